# revision 1
# baseline (speedup 1.0000x reference)
"""Trainium2 Bass kernel for nn_MetricLoss (segment_reduce / discriminative loss).

Reference math (K=32 labels, D=16):
  cents[s,k,:]  = mean of embeddings of sample s where label==k
  push[s]       = sum_{k<j} relu(0.25 - L1(c_sk, c_sj))^2 / 496
  pull[s]       = mean over ALL B*H*W pixels p of  L1(e_p, c_s,label_p)^2
  loss          = mean_s (push[s] + 0.1 * pull[s])

Strategy (8 cores, two launches, pixel-major layout [128 part x 576 cols]):
  Launch A: per-core partial centroid sums+counts.
    - one-hot(labels) in bf16 via DVE is_equal
    - PE: 84 groups; weights = 7 pixel-tiles of [emb(16) ; ones(1)] = 119 cols
      (padded to 128), rhs = the 7 tiles' one-hot [128, 224]; accumulated into
      a single PSUM bank; diagonal blocks hold per-tile-class sums+counts.
    - host sums blocks across groups/cores -> cents [4,32,16]
  Launch B: pull + push.
    - onehotT4 [128=(strip4,k32), 18432] via DVE tensor_scalar is_equal (4x)
    - gather: per pixel-tile MM with lhsT = onehotT4 strip slice [32,128],
      rhs = centsT [32,64] -> psum [128 px, 64=(b,d)] = gathered centroids
    - DVE: diff = psum - emb (b-broadcast), |.|-reduce over d -> dist [128,4,576]
    - DVE: per-b sum of dist^2 -> pacc [128,4] -> host reduce
    - push computed redundantly per core from tiny cent tables.
"""

import numpy as np
import ml_dtypes

import concourse.bass as bass
import concourse.bacc as bacc
import concourse.mybir as mybir
from concourse.tile import TileContext
from concourse.bass_utils import run_bass_kernel_spmd

BF16 = ml_dtypes.bfloat16
F32 = np.float32

# problem constants (hardcoded per contract)
B, H, W, D, K = 4, 384, 384, 16, 32
NCORES = 8
NPIX_TOT = B * H * W              # 589824
NPIX = NPIX_TOT // NCORES         # 73728 per core
P = 128                           # partitions
TC = NPIX // P                    # 576 pixel columns per partition
TCP = 588                         # padded to 7*84 for launch A grouping
NG = TCP // 7                     # 84 weight groups
GW = 7 * 17                       # 119 weight cols per group
WCOLS = GW * (NG - 1) + 128       # 10005 -> pad
WCOLS_PAD = 10016
NB = TC // 8                      # 72 gather batches (8 tiles each)
QS = TC // 4                      # 144 tiles per strip
LAB_PAD = 100.0                   # pad label (!= any of 0..31)

PUSH_MARGIN = 0.25
PUSH_W = 1.0
PULL_W = 0.1
NCMP = K * (K - 1) / 2.0

_built = {}


def _build_launch_a():
    nc = bacc.Bacc("TRN2", target_bir_lowering=False, debug=False)
    bf = mybir.dt.bfloat16
    f32 = mybir.dt.float32

    emb17 = nc.dram_tensor("emb17", [P, WCOLS_PAD], bf, kind="ExternalInput")
    labels = nc.dram_tensor("labels", [P, TCP], bf, kind="ExternalInput")
    outA = nc.dram_tensor("outA", [P, 224], f32, kind="ExternalOutput")

    with TileContext(nc) as tc:
        with (
            tc.tile_pool(name="sbuf", bufs=1) as pool,
            tc.tile_pool(name="psum", bufs=1, space="PSUM") as psum_pool,
        ):
            emb_sb = pool.tile([P, WCOLS_PAD], bf)
            lab_sb = pool.tile([P, TCP], bf)
            onehot = pool.tile([P, K, TCP], bf)  # k-major: 4x-mode generation

            nc.sync.dma_start(out=lab_sb[:], in_=labels.ap())
            # emb17 in 4 chunks for DMA/PE overlap
            NCH = 4
            ch = WCOLS_PAD // NCH
            for i in range(NCH):
                nc.sync.dma_start(
                    out=emb_sb[:, i * ch : (i + 1) * ch],
                    in_=emb17.ap()[:, i * ch : (i + 1) * ch],
                )

            # one-hot: per-k tensor_scalar is_equal (single-src bf16 -> 4x mode)
            half = TCP // 2
            for h in range(2):
                sl = slice(h * half, (h + 1) * half)
                for k in range(K):
                    nc.vector.tensor_scalar(
                        out=onehot[:, k, sl],
                        in0=lab_sb[:, sl],
                        scalar1=float(k),
                        scalar2=None,
                        op0=mybir.AluOpType.is_equal,
                    )

            ps = psum_pool.tile([P, 7, K], mybir.dt.float32)
            for g in range(NG):
                nc.tensor.matmul(
                    ps[:],
                    emb_sb[:, GW * g : GW * g + 128],
                    onehot[:, :, 7 * g : 7 * g + 7].rearrange("p k t -> p t k"),
                    start=(g == 0),
                    stop=(g == NG - 1),
                )

            evac = pool.tile([P, 7 * K], f32)
            nc.vector.tensor_copy(out=evac[:], in_=ps[:].rearrange("p a b -> p (a b)"))
            nc.sync.dma_start(out=outA.ap(), in_=evac[:])
    nc.compile()
    return nc


def _build_launch_b():
    nc = bacc.Bacc("TRN2", target_bir_lowering=False, debug=False)
    bf = mybir.dt.bfloat16
    f32 = mybir.dt.float32

    emb16 = nc.dram_tensor("emb16", [P, TC * D], bf, kind="ExternalInput")
    lab4 = nc.dram_tensor("lab4", [P, QS * P], bf, kind="ExternalInput")
    iotaT = nc.dram_tensor("iotaT", [P, 1], f32, kind="ExternalInput")
    centsT = nc.dram_tensor("centsT", [P, 64], bf, kind="ExternalInput")
    cpp = nc.dram_tensor("cpp", [P, D], bf, kind="ExternalInput")
    cjd = nc.dram_tensor("cjd", [P, K * D], bf, kind="ExternalInput")
    triu = nc.dram_tensor("triu", [P, K], bf, kind="ExternalInput")
    pacc_d = nc.dram_tensor("pacc", [P, 4], f32, kind="ExternalOutput")
    pushp_d = nc.dram_tensor("pushp", [P, 1], f32, kind="ExternalOutput")

    with TileContext(nc) as tc:
        with (
            tc.tile_pool(name="sbuf", bufs=1) as pool,
            tc.tile_pool(name="work", bufs=3) as wpool,
            tc.tile_pool(name="psum", bufs=2, space="PSUM") as psum_pool,
        ):
            emb_sb = pool.tile([P, TC, D], bf)
            lab4_sb = pool.tile([P, QS * P], bf)
            iotaT_sb = pool.tile([P, 1], f32)
            centsT_sb = pool.tile([P, 64], bf)
            cpp_sb = pool.tile([P, D], bf)
            cjd_sb = pool.tile([P, K, D], bf)
            triu_sb = pool.tile([P, K], bf)
            oh4 = pool.tile([P, QS * P], bf)
            dist = pool.tile([P, TC, 4], bf)  # t-major, b-inner (2x-mode APs)
            pacc = pool.tile([P, 4], f32)
            pushp = pool.tile([P, 1], f32)

            nc.sync.dma_start(out=iotaT_sb[:], in_=iotaT.ap())
            nc.sync.dma_start(out=centsT_sb[:], in_=centsT.ap())
            nc.sync.dma_start(out=cpp_sb[:], in_=cpp.ap())
            nc.sync.dma_start(out=cjd_sb[:], in_=cjd.ap().rearrange("p (a b) -> p a b", b=D))
            nc.sync.dma_start(out=triu_sb[:], in_=triu.ap())

            NCH = 4
            ech = TC // NCH
            for i in range(NCH):
                nc.sync.dma_start(
                    out=emb_sb[:, i * ech : (i + 1) * ech, :],
                    in_=emb16.ap().rearrange("p (t d) -> p t d", d=D)[
                        :, i * ech : (i + 1) * ech, :
                    ],
                )
            lch = (QS * P) // NCH
            for i in range(NCH):
                nc.sync.dma_start(
                    out=lab4_sb[:, i * lch : (i + 1) * lch],
                    in_=lab4.ap()[:, i * lch : (i + 1) * lch],
                )

            # one-hot (transposed, 4 strips) via tensor_scalar is_equal (4x mode)
            NOH = 12
            oc = (QS * P) // NOH
            for i in range(NOH):
                sl = slice(i * oc, (i + 1) * oc)
                nc.vector.tensor_scalar(
                    out=oh4[:, sl],
                    in0=lab4_sb[:, sl],
                    scalar1=iotaT_sb[:, 0:1],
                    scalar2=None,
                    op0=mybir.AluOpType.is_equal,
                )

            # gather + pull distance; superbatches of 32 tiles, one PSUM bank
            # per strip (concurrent row-strip MMs must hit distinct banks).
            # MM emission interleaves strips so next LDW overlaps current MM.
            NSB = TC // 32
            for sb in range(NSB):
                t0 = 32 * sb
                pss = [
                    psum_pool.tile(
                        [P, 8, 4, D], mybir.dt.float32, tag=f"ps{s}",
                        name=f"ps{s}_{sb}",
                    )
                    for s in range(4)
                ]
                for j in range(8):
                    for s in range(4):
                        q = 8 * sb + j
                        nc.tensor.matmul(
                            pss[s][:, j, :, :].rearrange("p a b -> p (a b)"),
                            oh4[32 * s : 32 * s + 32, P * q : P * (q + 1)],
                            centsT_sb[32 * s : 32 * s + 32, :],
                            start=True,
                            stop=True,
                            tile_position=(32 * s, 0),
                        )
                for s in range(4):
                    gev = wpool.tile([P, 8, 4, D], bf, tag=f"gev{s}")
                    nc.scalar.copy(out=gev[:], in_=pss[s][:])
                    diff = wpool.tile([P, 8, 4, D], bf, tag=f"diff{s}")
                    nc.vector.tensor_tensor(
                        out=diff[:],
                        in0=gev[:],
                        in1=emb_sb[:, t0 + s : t0 + s + 29 : 4, :]
                        .unsqueeze(2)
                        .broadcast_to([P, 8, 4, D]),
                        op=mybir.AluOpType.subtract,
                    )
                    with nc.allow_low_precision("dist in bf16; error averages out"):
                        nc.vector.tensor_reduce(
                            out=dist[:, t0 + s : t0 + s + 29 : 4, :],
                            in_=diff[:],
                            axis=mybir.AxisListType.X,
                            op=mybir.AluOpType.add,
                            apply_absolute_value=True,
                        )

            # pull partial: pacc[p, b] = sum_t dist^2
            sq = pool.tile([P, TC, 4], f32)
            nc.vector.tensor_tensor(
                out=sq[:], in0=dist[:], in1=dist[:], op=mybir.AluOpType.mult
            )
            nc.vector.tensor_reduce(
                out=pacc[:],
                in_=sq[:].rearrange("p t b -> p b t"),
                axis=mybir.AxisListType.X,
                op=mybir.AluOpType.add,
            )
            nc.sync.dma_start(out=pacc_d.ap(), in_=pacc[:])

            # push (tiny, redundant per core): partitions p=(b,k)
            pd_diff = pool.tile([P, K, D], bf)
            nc.vector.tensor_tensor(
                out=pd_diff[:],
                in0=cpp_sb[:].unsqueeze(1).broadcast_to([P, K, D]),
                in1=cjd_sb[:],
                op=mybir.AluOpType.subtract,
            )
            pd = pool.tile([P, K], f32)
            nc.vector.tensor_reduce(
                out=pd[:],
                in_=pd_diff[:],
                axis=mybir.AxisListType.X,
                op=mybir.AluOpType.add,
                apply_absolute_value=True,
            )
            # relu(margin - d)^2 == min(d - margin, 0)^2
            m = pool.tile([P, K], f32)
            nc.vector.tensor_scalar(
                out=m[:],
                in0=pd[:],
                scalar1=PUSH_MARGIN,
                scalar2=0.0,
                op0=mybir.AluOpType.subtract,
                op1=mybir.AluOpType.min,
            )
            msq = pool.tile([P, K], f32)
            nc.vector.tensor_tensor(
                out=msq[:], in0=m[:], in1=m[:], op=mybir.AluOpType.mult
            )
            msqm = pool.tile([P, K], f32)
            nc.vector.tensor_tensor(
                out=msqm[:], in0=msq[:], in1=triu_sb[:], op=mybir.AluOpType.mult
            )
            nc.vector.tensor_reduce(
                out=pushp[:],
                in_=msqm[:],
                axis=mybir.AxisListType.X,
                op=mybir.AluOpType.add,
            )
            nc.sync.dma_start(out=pushp_d.ap(), in_=pushp[:])
    nc.compile()
    return nc


def _get(name):
    if name not in _built:
        if name == "A":
            _built[name] = _build_launch_a()
        else:
            _built[name] = _build_launch_b()
    return _built[name]


def _prep_a(emb_flat, lab_flat):
    """emb_flat [NPIX_TOT, D] f32, lab_flat [NPIX_TOT] i32 -> per-core in_maps."""
    in_maps = []
    for c in range(NCORES):
        e = emb_flat[c * NPIX : (c + 1) * NPIX].astype(BF16).reshape(P, TC, D)
        l = lab_flat[c * NPIX : (c + 1) * NPIX].reshape(P, TC)
        e17 = np.zeros((P, TCP, 17), dtype=BF16)
        e17[:, :TC, :D] = e
        e17[:, :, D] = BF16(1.0)
        w = np.zeros((P, WCOLS_PAD), dtype=BF16)
        w[:, : TCP * 17] = e17.reshape(P, TCP * 17)
        lb = np.full((P, TCP), LAB_PAD, dtype=BF16)
        lb[:, :TC] = l.astype(BF16)
        in_maps.append({"emb17": w, "labels": lb})
    return in_maps


def _reduce_a(results):
    """outA [8][P, 224] -> cents [B, K, D] float64, counts [B, K]."""
    sums = np.zeros((B, K, D), dtype=np.float64)
    cnts = np.zeros((B, K), dtype=np.float64)
    for c in range(NCORES):
        o = results[c]["outA"].astype(np.float64).reshape(P, 7, K)
        s = c // 2
        for j in range(7):
            blk = o[17 * j : 17 * j + 17, j, :]  # [17, K]
            sums[s] += blk[:D].T  # [K, D]
            cnts[s] += blk[D]
    cents = sums / np.maximum(cnts, 1.0)[:, :, None]
    cents = np.where(cnts[:, :, None] > 0, cents, 0.0)
    return cents, cnts


def _prep_b(emb_flat, lab_flat, cents):
    iotaT = (np.arange(P, dtype=F32) % K).astype(F32).reshape(P, 1)
    centsT = np.zeros((P, 64), dtype=BF16)
    cb = cents.astype(F32)  # [B, K, D]
    for s in range(4):
        # centsT[32s+k, 16b+d] = cents[b, k, d]
        centsT[32 * s : 32 * s + 32, :] = (
            cb.transpose(1, 0, 2).reshape(K, 64).astype(BF16)
        )
    cpp = cb.reshape(P, D).astype(BF16)  # p = 32b + k
    cjd = np.zeros((P, K * D), dtype=BF16)
    for b in range(4):
        cjd[32 * b : 32 * b + 32, :] = np.broadcast_to(
            cb[b].reshape(1, K * D), (K, K * D)
        ).astype(BF16)
    triu = np.zeros((P, K), dtype=BF16)
    kk = np.arange(K)
    for b in range(4):
        triu[32 * b : 32 * b + 32, :] = (kk[None, :] > kk[:, None]).astype(BF16)

    in_maps = []
    for c in range(NCORES):
        e = emb_flat[c * NPIX : (c + 1) * NPIX].astype(BF16).reshape(P, TC, D)
        l = lab_flat[c * NPIX : (c + 1) * NPIX].reshape(P, TC)  # [m, tau]
        lab4 = np.zeros((P, QS * P), dtype=BF16)
        for s in range(4):
            a = l[:, s::4]  # [m, q]
            lab4[32 * s : 32 * s + 32, :] = np.broadcast_to(
                a.T.reshape(1, QS * P), (K, QS * P)
            ).astype(BF16)
        in_maps.append(
            {
                "emb16": e.reshape(P, TC * D),
                "lab4": lab4,
                "iotaT": iotaT.copy(),
                "centsT": centsT.copy(),
                "cpp": cpp.copy(),
                "cjd": cjd.copy(),
                "triu": triu.copy(),
            }
        )
    return in_maps


def run_launches(embeddings, labels, trace=False, trace_kwargs=None):
    """Returns (loss_scalar, resA, resB) — resA/resB are BassKernelResults."""
    emb_flat = np.ascontiguousarray(np.asarray(embeddings), dtype=F32).reshape(
        NPIX_TOT, D
    )
    lab_flat = np.ascontiguousarray(np.asarray(labels), dtype=np.int32).reshape(
        NPIX_TOT
    )
    core_ids = list(range(NCORES))

    kwA = dict(trace=trace, **(trace_kwargs or {}))
    resA = run_bass_kernel_spmd(_get("A"), _prep_a(emb_flat, lab_flat), core_ids, **kwA)
    cents, _ = _reduce_a(resA.results)

    resB = run_bass_kernel_spmd(
        _get("B"), _prep_b(emb_flat, lab_flat, cents), core_ids, **kwA
    )
    pull = np.zeros(4, dtype=np.float64)
    for c in range(NCORES):
        pull += resB.results[c]["pacc"].astype(np.float64).sum(axis=0)
    pull /= NPIX_TOT

    pushp = resB.results[0]["pushp"].astype(np.float64).reshape(4, K).sum(axis=1)
    push = pushp / NCMP

    loss = np.mean(PUSH_W * push + PULL_W * pull)
    return np.array(loss, dtype=F32), resA, resB


def kernel(embeddings, labels):
    loss, _, _ = run_launches(embeddings, labels, trace=False)
    return loss



# revision 2
# speedup vs baseline: 1.7989x; 1.7989x over previous
"""Trainium2 Bass kernel for nn_MetricLoss (segment_reduce / discriminative loss).

Reference math (K=32 labels, D=16):
  cents[s,k,:]  = mean of embeddings of sample s where label==k
  push[s]       = sum_{k<j} relu(0.25 - L1(c_sk, c_sj))^2 / 496
  pull[s]       = mean over ALL B*H*W pixels p of  L1(e_p, c_s,label_p)^2
  loss          = mean_s (push[s] + 0.1 * pull[s])

Strategy (8 cores, two launches):
  Launch A: per-core partial centroid sums+counts.
    - host precomputes one-hot in (group, tile, k) layout -> contiguous
      moving operand for the PE (strided rhs APs measured 3x slower)
    - PE: 84 groups; stationary = 7 pixel-tiles of [emb(16) ; ones(1)]
      (119 cols padded to 128), moving = the group's one-hot [128, 224];
      accumulated into one PSUM bank; diag blocks hold per-tile sums+counts.
    - host sums blocks across groups/cores -> cents [4,32,16]
  Launch B: pull + push.
    - PE computes diff = cents[b, label_p, :] - emb_p DIRECTLY:
      lhsT = BT[:, 128j:128j+128] with 96 contraction rows =
        [oh_A(32) ; embT_A(16) ; oh_B(32) ; embT_B(16)]  (two pixel halves)
      rhs  = rhsC [96, 128]: cols 0-63  = [centsT ; -I16x4 ; 0]  (half A)
                             cols 64-127= [0 ; centsT ; -I16x4]  (half B)
      -> psum [128 pix, (u2, b4, d16)] = diff, one MM per 128 pixels.
    - evac+|.|: mix of ACT Abs (psum->sbuf bf16) + DVE add-tree over d,
      and a few pairs via DVE tensor_reduce(abs) directly from psum,
      to balance ACT vs DVE load.
    - tail: dist^2 (TT 2x) + reduce over pixels -> pacc2 [128, 2, 4]
    - push computed redundantly per core from tiny cent tables.
"""

import numpy as np
import ml_dtypes

import concourse.bass as bass
import concourse.bacc as bacc
import concourse.mybir as mybir
from concourse.tile import TileContext
from concourse.bass_utils import run_bass_kernel_spmd

BF16 = ml_dtypes.bfloat16
F32 = np.float32

# problem constants (hardcoded per contract)
B, H, W, D, K = 4, 384, 384, 16, 32
NCORES = 8
NPIX_TOT = B * H * W              # 589824
NPIX = NPIX_TOT // NCORES         # 73728 per core
P = 128                           # partitions
TC = NPIX // P                    # 576 pixel columns per partition
TCP = 588                         # padded to 7*84 for launch A grouping
NG = TCP // 7                     # 84 weight groups
GW = 7 * 17                       # 119 weight cols per group
WCOLS_PAD = 10016                 # GW*(NG-1) + 128 = 10005 -> pad
LAB_PAD = 100.0                   # pad label (!= any of 0..31)

# launch B geometry
NHALF = NPIX // 2                 # 36864 pixels per half
NJ = NHALF // P                   # 288 blocks of 128 pixels per half
NPAIR = NJ // 8                   # 36 psum double-banks (8 blocks each)
DVE_PAIRS = {2, 9, 16, 23, 30}    # pairs evacuated via DVE reduce-direct

PUSH_MARGIN = 0.25
PUSH_W = 1.0
PULL_W = 0.1
NCMP = K * (K - 1) / 2.0

_built = {}


def _build_launch_a():
    nc = bacc.Bacc("TRN2", target_bir_lowering=False, debug=False)
    bf = mybir.dt.bfloat16
    f32 = mybir.dt.float32

    emb17 = nc.dram_tensor("emb17", [P, WCOLS_PAD], bf, kind="ExternalInput")
    ohA = nc.dram_tensor("ohA", [P, NG * 224], bf, kind="ExternalInput")
    outA = nc.dram_tensor("outA", [P, 224], f32, kind="ExternalOutput")

    with TileContext(nc) as tc:
        with (
            tc.tile_pool(name="sbuf", bufs=1) as pool,
            tc.tile_pool(name="psum", bufs=1, space="PSUM") as psum_pool,
        ):
            emb_sb = pool.tile([P, WCOLS_PAD], bf)
            oh_sb = pool.tile([P, NG * 224], bf)

            # chunked loads so PE can start after the first chunks land
            NCH = 4
            och = (NG * 224) // NCH
            ech = WCOLS_PAD // NCH
            for i in range(NCH):
                nc.sync.dma_start(
                    out=oh_sb[:, i * och : (i + 1) * och],
                    in_=ohA.ap()[:, i * och : (i + 1) * och],
                )
                nc.sync.dma_start(
                    out=emb_sb[:, i * ech : (i + 1) * ech],
                    in_=emb17.ap()[:, i * ech : (i + 1) * ech],
                )

            ps = psum_pool.tile([P, 7, K], mybir.dt.float32)
            for g in range(NG):
                nc.tensor.matmul(
                    ps[:],
                    emb_sb[:, GW * g : GW * g + 128],
                    oh_sb[:, 224 * g : 224 * (g + 1)],
                    start=(g == 0),
                    stop=(g == NG - 1),
                )

            evac = pool.tile([P, 7 * K], f32)
            nc.vector.tensor_copy(out=evac[:], in_=ps[:].rearrange("p a b -> p (a b)"))
            nc.sync.dma_start(out=outA.ap(), in_=evac[:])
    nc.compile()
    return nc


def _build_launch_b():
    nc = bacc.Bacc("TRN2", target_bir_lowering=False, debug=False)
    bf = mybir.dt.bfloat16
    f32 = mybir.dt.float32

    BTd = nc.dram_tensor("BT", [96, NJ * P], bf, kind="ExternalInput")
    rhsCd = nc.dram_tensor("rhsC", [96, 128], bf, kind="ExternalInput")
    cppd = nc.dram_tensor("cpp", [P, D], bf, kind="ExternalInput")
    cjdd = nc.dram_tensor("cjd", [P, K * D], bf, kind="ExternalInput")
    triud = nc.dram_tensor("triu", [P, K], bf, kind="ExternalInput")
    pacc2d = nc.dram_tensor("pacc2", [P, 8], f32, kind="ExternalOutput")
    pushpd = nc.dram_tensor("pushp", [P, 1], f32, kind="ExternalOutput")

    with TileContext(nc) as tc:
        with (
            tc.tile_pool(name="sbuf", bufs=1) as pool,
            tc.tile_pool(name="work", bufs=4) as wpool,
            tc.tile_pool(name="psum", bufs=4, space="PSUM") as psum_pool,
        ):
            BT = pool.tile([96, NJ * P], bf)
            rhsC = pool.tile([96, 128], bf)
            cpp_sb = pool.tile([P, D], bf)
            cjd_sb = pool.tile([P, K, D], bf)
            triu_sb = pool.tile([P, K], bf)
            dist = pool.tile([P, NJ, 2, 4], bf)
            sqb = pool.tile([P, NJ, 2, 4], bf)
            pacc2 = pool.tile([P, 2, 4], f32)
            pushp = pool.tile([P, 1], f32)

            nc.sync.dma_start(out=rhsC[:], in_=rhsCd.ap())
            nc.sync.dma_start(out=cpp_sb[:], in_=cppd.ap())
            nc.sync.dma_start(
                out=cjd_sb[:], in_=cjdd.ap().rearrange("p (a b) -> p a b", b=D)
            )
            nc.sync.dma_start(out=triu_sb[:], in_=triud.ap())

            NCH = 8
            ch = (NJ * P) // NCH
            for i in range(NCH):
                nc.sync.dma_start(
                    out=BT[:, i * ch : (i + 1) * ch],
                    in_=BTd.ap()[:, i * ch : (i + 1) * ch],
                )

            for i in range(NPAIR):
                ps = psum_pool.tile(
                    [P, 8, 128], mybir.dt.float32, tag="ps", name=f"ps_{i}"
                )
                for jj in range(8):
                    j = 8 * i + jj
                    nc.tensor.matmul(
                        ps[:, jj, :],
                        BT[:, P * j : P * (j + 1)],
                        rhsC[:],
                        start=True,
                        stop=True,
                    )
                dsl = dist[:, 8 * i : 8 * (i + 1), :, :]
                if i in DVE_PAIRS:
                    with nc.allow_low_precision("dist in bf16; error averages out"):
                        nc.vector.tensor_reduce(
                            out=dsl,
                            in_=ps[:].rearrange("p a (c d) -> p (a c) d", d=D),
                            axis=mybir.AxisListType.X,
                            op=mybir.AluOpType.add,
                            apply_absolute_value=True,
                        )
                else:
                    absd = wpool.tile([P, 64, 16], bf, tag="absd", name=f"absd_{i}")
                    nc.scalar.activation(
                        absd[:].rearrange("p a b -> p (a b)"),
                        ps[:].rearrange("p a b -> p (a b)"),
                        mybir.ActivationFunctionType.Abs,
                    )
                    t8 = wpool.tile([P, 64, 8], bf, tag="t8", name=f"t8_{i}")
                    nc.vector.tensor_tensor(
                        out=t8[:], in0=absd[:, :, 0:8], in1=absd[:, :, 8:16],
                        op=mybir.AluOpType.add,
                    )
                    t4 = wpool.tile([P, 64, 4], bf, tag="t4", name=f"t4_{i}")
                    nc.vector.tensor_tensor(
                        out=t4[:], in0=t8[:, :, 0:4], in1=t8[:, :, 4:8],
                        op=mybir.AluOpType.add,
                    )
                    t2 = wpool.tile([P, 64, 2], bf, tag="t2", name=f"t2_{i}")
                    nc.vector.tensor_tensor(
                        out=t2[:], in0=t4[:, :, 0:2], in1=t4[:, :, 2:4],
                        op=mybir.AluOpType.add,
                    )
                    nc.vector.tensor_tensor(
                        out=dsl, in0=t2[:, :, 0:1], in1=t2[:, :, 1:2],
                        op=mybir.AluOpType.add,
                    )

            # pull partial: pacc2[p, u, b] = sum_j dist^2
            nc.vector.tensor_tensor(
                out=sqb[:], in0=dist[:], in1=dist[:], op=mybir.AluOpType.mult
            )
            nc.vector.tensor_reduce(
                out=pacc2[:],
                in_=sqb[:].rearrange("p j u b -> p u b j"),
                axis=mybir.AxisListType.X,
                op=mybir.AluOpType.add,
            )
            nc.sync.dma_start(out=pacc2d.ap(), in_=pacc2[:].rearrange("p a b -> p (a b)"))

            # push (tiny, redundant per core): partitions p=(b,k)
            pd_diff = pool.tile([P, K, D], bf)
            nc.vector.tensor_tensor(
                out=pd_diff[:],
                in0=cpp_sb[:].unsqueeze(1).broadcast_to([P, K, D]),
                in1=cjd_sb[:],
                op=mybir.AluOpType.subtract,
            )
            pd = pool.tile([P, K], f32)
            nc.vector.tensor_reduce(
                out=pd[:],
                in_=pd_diff[:],
                axis=mybir.AxisListType.X,
                op=mybir.AluOpType.add,
                apply_absolute_value=True,
            )
            # relu(margin - d)^2 == min(d - margin, 0)^2
            m = pool.tile([P, K], f32)
            nc.vector.tensor_scalar(
                out=m[:],
                in0=pd[:],
                scalar1=PUSH_MARGIN,
                scalar2=0.0,
                op0=mybir.AluOpType.subtract,
                op1=mybir.AluOpType.min,
            )
            msq = pool.tile([P, K], f32)
            nc.vector.tensor_tensor(
                out=msq[:], in0=m[:], in1=m[:], op=mybir.AluOpType.mult
            )
            msqm = pool.tile([P, K], f32)
            nc.vector.tensor_tensor(
                out=msqm[:], in0=msq[:], in1=triu_sb[:], op=mybir.AluOpType.mult
            )
            nc.vector.tensor_reduce(
                out=pushp[:],
                in_=msqm[:],
                axis=mybir.AxisListType.X,
                op=mybir.AluOpType.add,
            )
            nc.sync.dma_start(out=pushpd.ap(), in_=pushp[:])
    nc.compile()
    return nc


def _get(name):
    if name not in _built:
        if name == "A":
            _built[name] = _build_launch_a()
        else:
            _built[name] = _build_launch_b()
    return _built[name]


def _prep_a(emb_flat, lab_flat):
    """emb_flat [NPIX_TOT, D] f32, lab_flat [NPIX_TOT] i32 -> per-core in_maps."""
    kk = np.arange(K, dtype=np.int32)
    in_maps = []
    for c in range(NCORES):
        e = emb_flat[c * NPIX : (c + 1) * NPIX].astype(BF16).reshape(P, TC, D)
        l = lab_flat[c * NPIX : (c + 1) * NPIX].reshape(P, TC)
        e17 = np.zeros((P, TCP, 17), dtype=BF16)
        e17[:, :TC, :D] = e
        e17[:, :, D] = BF16(1.0)
        w = np.zeros((P, WCOLS_PAD), dtype=BF16)
        w[:, : TCP * 17] = e17.reshape(P, TCP * 17)
        lb = np.full((P, TCP), int(LAB_PAD), dtype=np.int32)
        lb[:, :TC] = l
        oh = (lb[:, :, None] == kk[None, None, :]).astype(BF16)  # [P, 588, 32]
        in_maps.append({"emb17": w, "ohA": np.ascontiguousarray(oh.reshape(P, NG * 224))})
    return in_maps


def _reduce_a(results):
    """outA [8][P, 224] -> cents [B, K, D] float64, counts [B, K]."""
    sums = np.zeros((B, K, D), dtype=np.float64)
    cnts = np.zeros((B, K), dtype=np.float64)
    for c in range(NCORES):
        o = results[c]["outA"].astype(np.float64).reshape(P, 7, K)
        s = c // 2
        for j in range(7):
            blk = o[17 * j : 17 * j + 17, j, :]  # [17, K]
            sums[s] += blk[:D].T  # [K, D]
            cnts[s] += blk[D]
    cents = sums / np.maximum(cnts, 1.0)[:, :, None]
    cents = np.where(cnts[:, :, None] > 0, cents, 0.0)
    return cents, cnts


def _prep_b(emb_flat, lab_flat, cents):
    cb = cents.astype(F32)  # [B, K, D]
    centsT = cb.transpose(1, 0, 2).reshape(K, B * D)  # [32, 64] row k, col 16b+d
    negI = -np.concatenate([np.eye(D, dtype=F32)] * B, axis=1)  # [16, 64]
    rhsC = np.zeros((96, 128), dtype=BF16)
    rhsC[0:32, 0:64] = centsT.astype(BF16)
    rhsC[32:48, 0:64] = negI.astype(BF16)
    rhsC[48:80, 64:128] = centsT.astype(BF16)
    rhsC[80:96, 64:128] = negI.astype(BF16)

    cpp = cb.reshape(P, D).astype(BF16)  # p = 32b + k
    cjd = np.zeros((P, K * D), dtype=BF16)
    for b in range(4):
        cjd[32 * b : 32 * b + 32, :] = np.broadcast_to(
            cb[b].reshape(1, K * D), (K, K * D)
        ).astype(BF16)
    triu = np.zeros((P, K), dtype=BF16)
    kk = np.arange(K)
    for b in range(4):
        triu[32 * b : 32 * b + 32, :] = (kk[None, :] > kk[:, None]).astype(BF16)

    karange = np.arange(K, dtype=np.int32)
    in_maps = []
    for c in range(NCORES):
        e = emb_flat[c * NPIX : (c + 1) * NPIX].astype(BF16).reshape(2, NHALF, D)
        l = lab_flat[c * NPIX : (c + 1) * NPIX].reshape(2, NHALF)
        BT = np.zeros((96, NJ * P), dtype=BF16)
        for u in range(2):
            base = 48 * u
            BT[base : base + 32] = (karange[:, None] == l[u][None, :]).astype(BF16)
            BT[base + 32 : base + 48] = np.ascontiguousarray(e[u].T)
        in_maps.append(
            {
                "BT": BT,
                "rhsC": rhsC.copy(),
                "cpp": cpp.copy(),
                "cjd": cjd.copy(),
                "triu": triu.copy(),
            }
        )
    return in_maps


def run_launches(embeddings, labels, trace=False, trace_kwargs=None):
    """Returns (loss_scalar, resA, resB) — resA/resB are BassKernelResults."""
    emb_flat = np.ascontiguousarray(np.asarray(embeddings), dtype=F32).reshape(
        NPIX_TOT, D
    )
    lab_flat = np.ascontiguousarray(np.asarray(labels), dtype=np.int32).reshape(
        NPIX_TOT
    )
    core_ids = list(range(NCORES))

    kwA = dict(trace=trace, **(trace_kwargs or {}))
    resA = run_bass_kernel_spmd(_get("A"), _prep_a(emb_flat, lab_flat), core_ids, **kwA)
    cents, _ = _reduce_a(resA.results)

    resB = run_bass_kernel_spmd(
        _get("B"), _prep_b(emb_flat, lab_flat, cents), core_ids, **kwA
    )
    pull = np.zeros(4, dtype=np.float64)
    for c in range(NCORES):
        pull += resB.results[c]["pacc2"].astype(np.float64).reshape(P, 2, 4).sum(
            axis=(0, 1)
        )
    pull /= NPIX_TOT

    pushp = resB.results[0]["pushp"].astype(np.float64).reshape(4, K).sum(axis=1)
    push = pushp / NCMP

    loss = np.mean(PUSH_W * push + PULL_W * pull)
    return np.array(loss, dtype=F32), resA, resB


def kernel(embeddings, labels):
    loss, _, _ = run_launches(embeddings, labels, trace=False)
    return loss
